# revision 16
# baseline (speedup 1.0000x reference)
"""Trainium2 Bass kernel for nn_CustomWeightedTensorProduct (e3nn-style weighted
tensor product, 5 paths, per-edge weights).

Feature-major dataflow (v2), pure data-parallel over edges on 8 cores:

  - Host prep (per core slice of 12544 edges): transpose inputs to
    feature-major [feature, edge] layout, pre-scale the 5 weight blocks by
    their path constants, cast to bf16. Weight features are permuted so each
    128-row chunk c = 2*path+h holds rows q = 8u+w' (w = 8h+w'), letting one
    replica tile serve both w-halves of a path.
  - Device per 512-edge group:
      replicas  rep(x_vec)[q] = x_vec[q//8] via PE matmul with 0/1 stationary
                REP8 (PSUM -> SBUF via ScalarE copy)
      products  P = W_chunk * rep(x_vec)        (DVE, bf16 2x)
      u-reduce  PE matmul with 0/1 stationary S_red_j (j = position within a
                32-row PSUM quadrant; per-element has_written accumulation)
      combine   out1 terms duplicated/routed by PE matmuls (DUP/FIN
                stationaries); per-edge s2 multipliers applied by one DVE
                multiply per duplicated block set
  - All stationary matrices are 0/+-1 constants shipped as an extra
    ExternalInput, loaded to SBUF once. PE-matmul bases are quadrant-aligned
    (hardware tile_position constraint), so X1 rows live at partition bases
    0/32/64/96 and stationaries carry explicit zero columns for padding.
  - b[u] = <s1_1[u,:], s2_1> computed on-device with the same machinery.

Host post: transpose the [80, zc] bf16 output (rows 0..47 = out1 (i,w)-major,
rows 64..79 = out0) back to [z, 64] fp32.
"""

import sys

if "/opt/trn_rl_repo" not in sys.path:
    sys.path.insert(0, "/opt/trn_rl_repo")

import numpy as np

Z_FULL = 100000
N_CORES = 8
ZC = 12800                  # edges per core (25 uniform groups of 512)
Z_PAD = ZC * N_CORES        # 102400
E_GRP = 512                 # edges per group (matmul moving free dim)
N_GRP = ZC // E_GRP         # 25

SQRT2 = 2.0 ** 0.5
SQRT3 = 3.0 ** 0.5
K0 = 1.0 / (32.0 ** 0.5)
K1 = 1.0 / (48.0 ** 0.5)
K3 = K0 / SQRT3
KD = K1 / SQRT2

# TA blocks (16 rows each): [t0u, t1, t2_0, t2_1, t2_2, t4_0, t4_1, t4_2]
A2B = [0, 1, 1, 1, 2, 3, 4, 6]   # TA2 = [t0u, t1,t1,t1, t2_0,t2_1,t2_2, t4_1]
B2B = [7, 5, 7, 5, 6]            # TB2 = [t4_2, t4_0, t4_2, t4_0, t4_1]
RA_ROWS = [0, 1, 2, 3, 0, 0, 0, 3]   # x2 row (0=s20, 1+i=s21_i) per TA2 block
RB_ROWS = [1, 2, 2, 3, 1]
RB_SIGN = [1, 1, -1, -1, -1]
# out1 block (0..2) per U block; TA2 block0 (t0u) routes to out0 separately
FINA2_ROWS = [None, 0, 1, 2, 0, 1, 2, 0]
FINB_ROWS = [1, 2, 0, 1, 2]

# const tensor column offsets
OFF_REP8 = 0        # [16, 128] at partition bases 0/32/64
OFF_REP8HI = 128    # [32, 128] at partition base 64 (maps rows 80..95)
OFF_SRED = 256      # 4 variants [128, 32]
OFF_REP21F = 384    # [4, 96]
OFF_SB64 = 480      # [64, 16]
OFF_DUPA = 496      # [128, 128]
OFF_DUPB = 624      # [128, 80]
OFF_RA = 704        # [4, 128]
OFF_RB = 832        # [4, 80]
OFF_FINA2 = 912     # [128, 64]  (cols 48..63 zero)
OFF_FINA1 = 976     # [128, 16]
OFF_FINB = 992      # [80, 48]
CST_COLS = 1040


def build_consts() -> np.ndarray:
    cst = np.zeros((128, CST_COLS), np.float32)
    for base in (0, 32, 64):
        for q in range(128):
            cst[base + q // 8, OFF_REP8 + q] = 1.0
    for q in range(128):
        cst[64 + 16 + q // 8, OFF_REP8HI + q] = 1.0
    for j in range(4):
        for q in range(128):
            cst[q, OFF_SRED + 32 * j + 8 * j + q % 8] = 1.0
    for u in range(16):
        cst[1, OFF_REP21F + 32 + u] = 1.0
        cst[2, OFF_REP21F + 64 + u] = 1.0
        cst[3, OFF_REP21F + 80 + u] = 1.0
    for r in range(64):
        if r < 16:
            cst[r, OFF_SB64 + r] = 1.0
        elif 32 <= r < 48:
            cst[r, OFF_SB64 + r - 32] = 1.0
        elif r >= 48:
            cst[r, OFF_SB64 + r - 48] = 1.0
    for q2 in range(128):
        cst[16 * A2B[q2 // 16] + q2 % 16, OFF_DUPA + q2] = 1.0
        cst[RA_ROWS[q2 // 16], OFF_RA + q2] = 1.0
        blk = q2 // 16
        if blk == 0:
            cst[q2, OFF_FINA1 + q2 % 16] = 1.0
        else:
            cst[q2, OFF_FINA2 + 16 * FINA2_ROWS[blk] + q2 % 16] = 1.0
    for q2 in range(80):
        cst[16 * B2B[q2 // 16] + q2 % 16, OFF_DUPB + q2] = 1.0
        cst[RB_ROWS[q2 // 16], OFF_RB + q2] = RB_SIGN[q2 // 16]
        cst[q2, OFF_FINB + 16 * FINB_ROWS[q2 // 16] + q2 % 16] = 1.0
    return cst


def build_bass(loop=None, n_groups=None):
    """loop=J wraps the group loop in a For_i executed J times (timing aid).
    n_groups limits the number of edge-groups (testing aid)."""
    import concourse.bass as bass  # noqa: F401
    import concourse.bacc as bacc
    import concourse.mybir as mybir
    from concourse.tile import TileContext

    f32 = mybir.dt.float32
    bf = mybir.dt.bfloat16
    MULT = mybir.AluOpType.mult

    groups = [(g * E_GRP, E_GRP) for g in range(N_GRP)]
    if n_groups is not None:
        groups = groups[:n_groups]

    nc = bacc.Bacc(None, target_bir_lowering=False)
    x1t_d = nc.dram_tensor("x1t", [96, ZC], bf, kind="ExternalInput")
    x2t_d = nc.dram_tensor("x2t", [4, ZC], bf, kind="ExternalInput")
    # group-blocked: [p, g, c, e] so each group load is one 10KB
    # contiguous run per partition (128 descriptors, not 1280)
    wt_d = nc.dram_tensor("wt", [128, 10 * ZC], bf, kind="ExternalInput")
    cst_d = nc.dram_tensor("cst", [128, CST_COLS], bf, kind="ExternalInput")
    out_d = nc.dram_tensor("out", [80, ZC], bf, kind="ExternalOutput")

    with TileContext(nc) as tc:
        with (
            tc.tile_pool(name="cst", bufs=1) as pc,
            tc.tile_pool(name="io", bufs=2) as pio,
            tc.tile_pool(name="mid", bufs=2) as pm,
            tc.tile_pool(name="pso", bufs=2, space=bass.MemorySpace.PSUM) as ppo,
            tc.tile_pool(name="psr", bufs=3, space=bass.MemorySpace.PSUM) as ppr,
            tc.tile_pool(name="ps1", bufs=1, space=bass.MemorySpace.PSUM) as pp1,
        ):
            CST = pc.tile([128, CST_COLS], bf)
            nc.sync.dma_start(CST[:], cst_d[:])
            REP8 = {b: CST[b:b + 16, OFF_REP8:OFF_REP8 + 128]
                    for b in (0, 32, 64)}
            REP8HI = CST[64:96, OFF_REP8HI:OFF_REP8HI + 128]
            SREDJ = [CST[0:128, OFF_SRED + 32 * j:OFF_SRED + 32 * (j + 1)]
                     for j in range(4)]
            REP21F = CST[0:4, OFF_REP21F:OFF_REP21F + 96]
            SB64 = CST[0:64, OFF_SB64:OFF_SB64 + 16]
            DUPA = CST[0:128, OFF_DUPA:OFF_DUPA + 128]
            DUPB = CST[0:128, OFF_DUPB:OFF_DUPB + 80]
            RAPAT = CST[0:4, OFF_RA:OFF_RA + 128]
            RBPAT = CST[0:4, OFF_RB:OFF_RB + 80]
            FINA2 = CST[0:128, OFF_FINA2:OFF_FINA2 + 64]
            FINA1 = CST[0:128, OFF_FINA1:OFF_FINA1 + 16]
            FINB = CST[0:80, OFF_FINB:OFF_FINB + 48]

            _loop_cm = tc.For_i(0, int(loop)) if loop is not None else None
            if _loop_cm is not None:
                _loop_cm.__enter__()

            for e0, E in groups:
                # ---- loads ----
                Wt = pio.tile([128, 10, E], bf)
                X1 = pio.tile([96, E], bf)
                X2 = pio.tile([4, E], bf)
                wv = wt_d[:, 10 * e0:10 * (e0 + E)].rearrange(
                    "p (c e) -> p c e", c=10)
                nc.sync.dma_start(Wt[:], wv)
                nc.gpsimd.dma_start(X1[:], x1t_d[:, e0:e0 + E])
                nc.gpsimd.dma_start(X2[:], x2t_d[:, e0:e0 + E])

                OUTP = ppo.tile([128, E], f32)

                # ---- b = sum_i s11_i * s21_i  -> OUTP rows 96..111 ----
                # R21p transiently occupies OUTP rows 0..95 (dead before t3/FIN)
                nc.tensor.matmul(OUTP[0:96, :], REP21F, X2[:],
                                 start=True, stop=True, skip_group_check=True)
                Pb = pm.tile([64, E], bf)
                nc.vector.tensor_tensor(Pb[0:32, :], X1[32:64, :],
                                        OUTP[32:64, :], MULT)
                nc.vector.tensor_tensor(Pb[32:64, :], X1[64:96, :],
                                        OUTP[64:96, :], MULT)
                nc.tensor.matmul(OUTP[96:112, :], SB64, Pb[:],
                                 start=True, stop=True, skip_group_check=True,
                                 tile_position=(0, 96))
                B16 = pm.tile([16, E], bf)
                nc.scalar.copy(B16[:], OUTP[96:112, :])

                # ---- replicas: R11_0..2, R10, Rb ----
                def make_rep(k, stat, src):
                    RP = ppr.tile([128, E], f32, name=f"RP{k}", tag="RP")
                    nc.tensor.matmul(RP[:], stat, src, start=True, stop=True)
                    RS = pm.tile([128, E], bf, name=f"RS{k}", tag=f"RS{k}")
                    nc.scalar.copy(RS[:], RP[:])
                    return RS

                R11 = [
                    make_rep(0, REP8[32], X1[32:48, :]),
                    make_rep(1, REP8[64], X1[64:80, :]),
                    make_rep(2, REP8HI, X1[64:96, :]),
                ]
                R10 = make_rep(3, REP8[0], X1[0:16, :])
                RB = make_rep(4, REP8[0], B16[:])

                # ---- products (DVE bf16 2x; one TT per chunk, stride-1 APs) ----
                P01 = pm.tile([128, 4, E], bf)
                for k in range(4):
                    nc.vector.tensor_tensor(
                        P01[:, k, :], Wt[:, k, :], R10[:], MULT)
                P3 = pm.tile([128, 2, E], bf)
                for h in range(2):
                    nc.vector.tensor_tensor(
                        P3[:, h, :], Wt[:, 6 + h, :], RB[:], MULT)
                P2 = []
                P4 = []
                for i in range(3):
                    t2 = pm.tile([128, 2, E], bf, name=f"P2_{i}", tag=f"P2_{i}")
                    for h in range(2):
                        nc.vector.tensor_tensor(
                            t2[:, h, :], Wt[:, 4 + h, :], R11[i][:], MULT)
                    P2.append(t2)
                    t4 = pm.tile([128, 2, E], bf, name=f"P4_{i}", tag=f"P4_{i}")
                    for h in range(2):
                        nc.vector.tensor_tensor(
                            t4[:, h, :], Wt[:, 8 + h, :], R11[i][:], MULT)
                    P4.append(t4)

                # ---- u-reduces into TA quadrants (j-ordered for LDW reuse) ----
                TA = pp1.tile([128, E], f32)
                QPROD = [
                    [P01[:, 0, :], P01[:, 1, :], P01[:, 2, :], P01[:, 3, :]],
                    [P2[0][:, 0, :], P2[0][:, 1, :],
                     P2[1][:, 0, :], P2[1][:, 1, :]],
                    [P2[2][:, 0, :], P2[2][:, 1, :],
                     P4[0][:, 0, :], P4[0][:, 1, :]],
                    [P4[1][:, 0, :], P4[1][:, 1, :],
                     P4[2][:, 0, :], P4[2][:, 1, :]],
                ]
                for j in range(4):
                    for q in range(4):
                        nc.tensor.matmul(
                            TA[32 * q:32 * (q + 1), :], SREDJ[j], QPROD[q][j],
                            start=(j == 0), stop=(j == 3),
                            skip_group_check=True,
                            tile_position=(0, 32 * q))
                    if j < 2:
                        # t3 -> OUTP rows 64..79 (quadrant 2)
                        nc.tensor.matmul(
                            OUTP[64:96, :], SREDJ[j], P3[:, j, :],
                            start=(j == 0), stop=False, skip_group_check=True,
                            tile_position=(0, 64))

                TAs = pm.tile([128, E], bf)
                nc.scalar.copy(TAs[:], TA[:])

                # ---- U_A pass (TAB2/RABp shared between A and B passes) ----
                TAB2 = pp1.tile([128, E], f32)
                RABp = pp1.tile([128, E], f32)
                nc.tensor.matmul(TAB2[:], DUPA, TAs[:], start=True, stop=True)
                nc.tensor.matmul(RABp[:], RAPAT, X2[:], start=True, stop=True)
                TA2s = pm.tile([128, E], bf)
                nc.scalar.copy(TA2s[:], TAB2[:])
                UA = pm.tile([128, E], bf)
                nc.vector.tensor_tensor(UA[:], TA2s[:], RABp[:], MULT)

                # ---- U_B pass ----
                nc.tensor.matmul(TAB2[0:80, :], DUPB, TAs[:],
                                 start=True, stop=True)
                nc.tensor.matmul(RABp[0:80, :], RBPAT, X2[:],
                                 start=True, stop=True)
                TB2s = pm.tile([80, E], bf)
                nc.scalar.copy(TB2s[:], TAB2[0:80, :])
                UB = pm.tile([80, E], bf)
                nc.vector.tensor_tensor(UB[:], TB2s[:], RABp[0:80, :], MULT)

                # ---- finals ----
                # rows 0..63: FINA2 clears (start=True, cols 48..63 zero)
                nc.tensor.matmul(OUTP[0:64, :], FINA2, UA[:],
                                 start=True, stop=False, skip_group_check=True)
                nc.tensor.matmul(OUTP[0:48, :], FINB, UB[:],
                                 start=False, stop=True, skip_group_check=True)
                # rows 64..79: accumulate U_A block0 onto t3
                nc.tensor.matmul(OUTP[64:80, :], FINA1, UA[:],
                                 start=False, stop=True, skip_group_check=True)

                OUTs = pm.tile([80, E], bf)
                nc.scalar.copy(OUTs[:], OUTP[0:80, :])
                nc.scalar.dma_start(out_d[:, e0:e0 + E], OUTs[:])

            if _loop_cm is not None:
                _loop_cm.__exit__(None, None, None)

    nc.compile()
    return nc


_CACHE = {}

# test-harness hooks (ignored by the grading path)
TRACE = False
LAST_RESULTS = None


def _get_nc():
    if "nc" not in _CACHE:
        _CACHE["nc"] = build_bass()
    return _CACHE["nc"]


def host_prep(x1, x2, w):
    """Full-batch host prep: returns per-core input maps."""
    import ml_dtypes
    bf16 = ml_dtypes.bfloat16

    x1 = np.asarray(x1, dtype=np.float32)
    x2 = np.asarray(x2, dtype=np.float32)
    w = np.asarray(w, dtype=np.float32)
    z = x1.shape[0]
    pad = Z_PAD - z
    if pad:
        x1 = np.pad(x1, ((0, pad), (0, 0)))
        x2 = np.pad(x2, ((0, pad), (0, 0)))
        w = np.pad(w, ((0, pad), (0, 0)))

    # x1T [96, Z]: s10 @ 0..15, s11_0 @ 32..47, s11_1 @ 64..79, s11_2 @ 80..95
    x1T = np.zeros((96, Z_PAD), np.float32)
    x1T[0:16] = x1[:, :16].T
    s11 = x1[:, 16:64].reshape(Z_PAD, 16, 3)
    x1T[32:48] = s11[:, :, 0].T
    x1T[64:80] = s11[:, :, 1].T
    x1T[80:96] = s11[:, :, 2].T
    x1T = x1T.astype(bf16)

    x2T = x2.T.astype(bf16)

    # wT [1280, Z]: chunk c=2p+h rows 8u+w' = w5[:, p, u, 8h+w'] * scale_p
    w5 = w.reshape(Z_PAD, 5, 16, 16)
    scales = np.array([K0, K1, K1, K3, KD], np.float32)
    w5 = w5 * scales[None, :, None, None]
    # [z, p, u, h, w'] -> chunk-major [c=(p,h), q=(u,w'), z]
    wT = np.ascontiguousarray(
        w5.reshape(Z_PAD, 5, 16, 2, 8).transpose(1, 3, 2, 4, 0)
    ).reshape(1280, Z_PAD).astype(bf16)

    cst = build_consts().astype(bf16)

    in_maps = []
    for k in range(N_CORES):
        s = slice(k * ZC, (k + 1) * ZC)
        # per-core W, group-blocked: [10c, 128p, 25g, 512e] -> [p, g, c, e]
        wtc = wT[:, s].reshape(10, 128, N_GRP, E_GRP)
        wtb = np.ascontiguousarray(
            wtc.transpose(1, 2, 0, 3)).reshape(128, 10 * ZC)
        in_maps.append({
            "x1t": np.ascontiguousarray(x1T[:, s]),
            "x2t": np.ascontiguousarray(x2T[:, s]),
            "wt": wtb,
            "cst": cst,
        })
    return in_maps


def host_post(res_list, z):
    """Assemble [z, 64] fp32 from per-core [80, ZC] bf16 outputs."""
    outT = np.concatenate([np.asarray(r["out"]) for r in res_list], axis=1)
    outT = outT.astype(np.float32)          # [80, Z_PAD]
    out = np.empty((Z_PAD, 64), np.float32)
    out[:, 0:16] = outT[64:80].T
    o1 = outT[0:48].reshape(3, 16, Z_PAD)   # (i, w, z) -> col 16 + 3w + i
    out[:, 16:64] = o1.transpose(2, 1, 0).reshape(Z_PAD, 48)
    return np.ascontiguousarray(out[:z])


def kernel(x1, x2, w):
    global LAST_RESULTS
    from concourse.bass_utils import run_bass_kernel_spmd

    z = np.asarray(x1).shape[0]
    in_maps = host_prep(x1, x2, w)
    nc = _get_nc()
    res = run_bass_kernel_spmd(
        nc, in_maps, core_ids=list(range(N_CORES)), trace=TRACE)
    LAST_RESULTS = res
    return host_post(res.results, z)
